# revision 3
# baseline (speedup 1.0000x reference)
"""Neural CDE (RK4) Trainium2 kernel, v2.

Changes vs v1 baseline:
- double-fp16 (value+residual) weights/activations, 3-term products
  (~10x more accurate than double-bf16-4-term, fewer matmuls).
- G=2 interleaved batch-group pipelines per core (8+8 of 16 rows) so the
  two serial RK4 dependency chains fill each other's latency bubbles.
- "direct-MM": the x-contraction of the einsum is folded into L1's
  matmul via a stride-0 PSUM output AP (fp32 exact), removing the
  reduce + state-cast ops from the critical chain.
- L1 bias folded into an augmented [w1;b1] stationary (K=65) acting on
  folded state; L2/L3 biases via cheap fp16 row-seed matmuls.
- relu outputs fp16 directly on Act; fp16 residuals via one DVE stt from
  PSUM; bookkeeping (reduce, state updates, kaccs) on GpSimd off-chain.
- single per-step wf DMA (host-precomputed [128,144] (j,c)-major tile),
  zall output DMAs issued from the Pool queue.
"""

import os
import sys
from contextlib import ExitStack

import numpy as np
import ml_dtypes

sys.path.insert(0, "/opt/trn_rl_repo")

import concourse.bass as bass
import concourse.tile as tile
from concourse import bacc
from concourse import mybir
from concourse.bass_utils import run_bass_kernel_spmd

B, L, X, Z, H = 128, 512, 16, 64, 128
NCORES = 8
BPC = B // NCORES  # 16
G = 2              # batch groups per core
NB = BPC // G      # 8
W = 9 * NB         # 72 wf/ge/m columns per group
DT = 0.1
F32 = mybir.dt.float32
F16 = mybir.dt.float16
AF = mybir.ActivationFunctionType
OP = mybir.AluOpType

DIRECT_MM = True
STRIDE9_MM = True
GPDMA = False
POOLMULT = True
DEBUG = False
F32R = False
PUMP = True

# x-major permutation: psum/ge position (p, c) holds original W3 column
# z*16+x with x = 2c + (p>=64), z = p%64  (same as v1)
_p = np.arange(128)
_c = np.arange(8)
ORIG_COL = (_p[None, :] % 64) * 16 + 2 * _c[:, None] + (_p[None, :] // 64)  # [8,128]

ALPHAS = [0.5 * DT, 0.5 * DT, DT, DT / 6.0]  # stage input scales + final


def build_nc(l_steps=L):
    nc = bacc.Bacc("TRN2")
    dp = nc.declare_dram_parameter

    # ---- DRAM parameters (per core) ----
    wf_d = dp("wf", [l_steps, 128, 144], F32, isOutput=False).ap()
    # fp16 packed weights: [w1dupb|w1dupr|w2b|w2r|w3b(1024)|w3r(1024)]
    wmm_d = dp("wmm", [128, 2560], F16, isOutput=False).ap()
    # w1dup * alpha fp32(/fp32r), 3 variants packed [128, 384]
    w1da_d = dp("w1da", [128, 384],
                mybir.dt.float32r if F32R else F32, isOutput=False).ap()
    # fp16 pairs of w1dup*alpha: 3 variants x (b|r) packed [128, 768]
    w1da16_d = dp("w1da16", [128, 768], F16, isOutput=False).ap()
    # stacked bias rows fp16: [2, b1pair(128) | b2pair(128)]
    b12_d = dp("b12", [2, 256], F16, isOutput=False).ap()
    # stacked b3 rows + sel16(stride9) + sel16(contig): [16, 128+72+64]
    b3s_d = dp("b3s", [16, 264], F16, isOutput=False).ap()
    ones_d = dp("onesr", [2, 16], F16, isOutput=False).ap()
    # init mlp (fp32)
    wi1x_d = dp("wi1x", [16, 144], F32, isOutput=False).ap()  # [wi1 | x0t]
    wi2_d = dp("wi2", [128, 128], F32, isOutput=False).ap()
    wi3_d = dp("wi3", [128, 64], F32, isOutput=False).ap()
    bi1_d = dp("bi1", [128, 1], F32, isOutput=False).ap()
    bi2_d = dp("bi2", [128, 1], F32, isOutput=False).ap()
    bi3_d = dp("bi3", [64, 1], F32, isOutput=False).ap()
    # split-form state per step per group; host folds halves
    zout = [dp(f"zall{g}", [l_steps, 128, NB], F32, isOutput=True).ap()
            for g in range(G)]
    if DEBUG:
        dbg_ph1 = dp("dbg_ph1", [4, 128, NB], F32, isOutput=True).ap()
        dbg_h1 = dp("dbg_h1", [2, 128, NB], F16, isOutput=True).ap()
        dbg_h2 = dp("dbg_h2", [2, 128, NB], F16, isOutput=True).ap()
        dbg_ge = dp("dbg_ge", [128, 9 * NB], F32, isOutput=True).ap()
        dbg_m = dp("dbg_m", [128, 9 * NB], F32, isOutput=True).ap()
        dbg_q = dp("dbg_q", [4, 128, NB], F32, isOutput=True).ap()

    with tile.TileContext(nc) as tc, ExitStack() as ctx:
        singles = ctx.enter_context(tc.tile_pool(name="singles", bufs=1))
        wfp = ctx.enter_context(tc.tile_pool(name="wfp", bufs=4))
        gep = ctx.enter_context(tc.tile_pool(name="gep", bufs=12))
        mp = ctx.enter_context(tc.tile_pool(name="mp", bufs=12))
        hp = ctx.enter_context(tc.tile_pool(name="hp", bufs=12))
        qp = ctx.enter_context(tc.tile_pool(name="qp", bufs=12))
        sp = ctx.enter_context(tc.tile_pool(name="sp", bufs=8))   # slot/pfin
        zfp = ctx.enter_context(tc.tile_pool(name="zfp", bufs=12))  # folded state
        php = ctx.enter_context(tc.tile_pool(name="php", bufs=1, space="PSUM"))
        gpp = ctx.enter_context(tc.tile_pool(name="gpp", bufs=2, space="PSUM"))

        def load(pool, ap):
            t = pool.tile(list(ap.shape), ap.dtype, tag=ap.tensor.name)
            nc.sync.dma_start(out=t[:], in_=ap)
            return t

        wmm = load(singles, wmm_d)
        w1db, w1dr = wmm[:, 0:128], wmm[:, 128:256]
        w2b, w2r = wmm[:, 256:384], wmm[:, 384:512]
        w3b, w3r = wmm[:, 512:1536], wmm[:, 1536:2560]
        w1da = load(singles, w1da_d)
        W1A = {0.05: w1da[:, 0:128], 0.1: w1da[:, 128:256],
               DT / 6.0: w1da[:, 256:384]}
        w1da16 = load(singles, w1da16_d)
        W1A16 = [(w1da16[:, i * 256:i * 256 + 128],
                  w1da16[:, i * 256 + 128:(i + 1) * 256]) for i in range(3)]
        b12 = load(singles, b12_d)
        b1row2, b2row2 = b12[:, 0:128], b12[:, 128:256]
        b3s = load(singles, b3s_d)
        b3row16, sel16 = b3s[:, 0:128], b3s[:, 128:200]
        sel16c = b3s[:, 200:264]
        onesr = load(singles, ones_d)
        wi1x = load(singles, wi1x_d)
        wi1, x0t = wi1x[:, 0:128], wi1x[:, 128:144]
        wi2 = load(singles, wi2_d)
        wi3 = load(singles, wi3_d)
        bi1 = load(singles, bi1_d)
        bi2 = load(singles, bi2_d)
        bi3 = load(singles, bi3_d)

        mm = nc.tensor.matmul

        def rep_out(ph, n_rep):
            """psum out AP writing the same NB cols n_rep times (accumulate)."""
            base = ph[:]
            return bass.AP(tensor=ph.tensor, offset=base.offset,
                           ap=[[base.ap[0][0], 128], [0, n_rep], [1, NB]])

        # ---- init MLP (fp32): z0 = mlp(x(t0)) for all 16 batch cols ----
        gi = gpp.tile([128, 144], F32, tag="gp")
        ph_i1 = gi[:, 0:BPC]
        mm(ph_i1, wi1, x0t, start=True, stop=True)
        hi1 = singles.tile([128, BPC], F32, tag="hi1")
        nc.scalar.activation(hi1[:], ph_i1, AF.Relu, bias=bi1[:])
        gi2 = gpp.tile([128, 144], F32, tag="gp")
        ph_i2 = gi2[:, 0:BPC]
        mm(ph_i2, wi2[:], hi1[:], start=True, stop=True)
        hi2 = singles.tile([128, BPC], F32, tag="hi2")
        nc.scalar.activation(hi2[:], ph_i2, AF.Relu, bias=bi2[:])
        gi3 = gpp.tile([128, 144], F32, tag="gp")
        ph_i3 = gi3[0:64, 0:BPC]
        mm(ph_i3, wi3[:], hi2[:], start=True, stop=True)
        z0full = singles.tile([128, BPC], F32, tag="z0full")
        nc.vector.tensor_scalar_add(z0full[0:64, :], ph_i3, bi3[:])
        nc.vector.memset(z0full[64:128, :], 0.0)

        # per-group recurrent state
        slot = [None] * G     # [128, NB] f32 split-form z_n
        zft = [None] * G      # [128, 2NB] f16 state pair tile
        zfb = [None] * G
        zfr = [None] * G
        ge_cur = [None] * G   # [128, W] f32, svd col pre-written
        m_prev = [None] * G   # [128, W] f32 last stage's product tensor
        q_prev = [None] * G   # [128, NB] f32 last stage's reduced k
        qpair_prev = [None] * G

        def pair_casts(g, src_slot, tag):
            """src_slot [128,NB] f32 -> fp16 (value|residual) in one tile."""
            ft = zfp.tile([128, 2 * NB], F16, tag=f"{tag}{g}")
            fb, fr = ft[:, 0:NB], ft[:, NB:2 * NB]
            nc.scalar.copy(out=fb, in_=src_slot[:])
            nc.vector.scalar_tensor_tensor(out=fr, in0=fb, scalar=-1.0,
                                           in1=src_slot[:], op0=OP.mult,
                                           op1=OP.add)
            return ft, fb, fr

        for g in range(G):
            s0 = sp.tile([128, NB], F32, tag=f"slot{g}")
            nc.vector.tensor_copy(out=s0[:], in_=z0full[:, g * NB:(g + 1) * NB])
            slot[g] = s0
            zft[g], zfb[g], zfr[g] = pair_casts(g, s0, "zf")
            geg = gep.tile([128, W], F32, tag=f"ge{g}")
            ge3 = geg[:].rearrange("p (j c) -> p j c", c=9)
            nc.vector.tensor_copy(out=ge3[:, :, 8], in_=s0[:])
            ge_cur[g] = geg

        for t in range(l_steps):
            wf = wfp.tile([128, 144], F32, tag="wf")
            nc.sync.dma_start(out=wf[:], in_=wf_d[t])

            for g in range(G):
                eng = nc.gpsimd if GPDMA else nc.sync
                eng.dma_start(out=zout[g][t], in_=slot[g][:])

            pfin = [None] * G
            pft = [None] * G
            pfb = [None] * G
            pfr = [None] * G
            kacc = [None] * G
            qs = [[None] * 4 for _ in range(G)]

            for s in range(4):
                gp_full = gpp.tile([128, 144], F32, tag="gp")
                for g in range(G):
                    wfg = wf[:, g * W:(g + 1) * W]
                    ge = ge_cur[g]

                    # ---- L1: base (split-form state) + b1 seed + direct MM
                    ph1 = php.tile([128, NB], F32, tag=f"ph1g{g}")
                    if s == 0 and t > 0:
                        bt, bb, br = pft_prev[g], pfb_prev[g], pfr_prev[g]
                    else:
                        bt, bb, br = zft[g], zfb[g], zfr[g]
                    mm(ph1[:], b1row2, onesr[:, 0:NB], start=True, stop=False,
                       skip_group_check=True)
                    last_base = t == 0 and s == 0
                    if PUMP:
                        mv2 = bt[:].rearrange("p (k j) -> p k j", j=NB)
                        mm(rep_out(ph1, 2), w1db, mv2, start=False, stop=False,
                           skip_group_check=True)
                    else:
                        mm(ph1[:], w1db, bb, start=False, stop=False,
                           skip_group_check=True)
                        mm(ph1[:], w1db, br, start=False, stop=False,
                           skip_group_check=True)
                    mm(ph1[:], w1dr, bb, start=False, stop=last_base,
                       skip_group_check=True)
                    if not last_base:
                        alpha = ALPHAS[s - 1] if s > 0 else DT / 6.0
                        if DIRECT_MM:
                            # rhs = m_prev viewed (c outer, j inner); out
                            # stride-0 over c accumulates the x-reduce.
                            mv = m_prev[g][:].rearrange("p (j c) -> p c j",
                                                        c=9)
                            if F32R:
                                mv = mv.bitcast(mybir.dt.float32r)
                            out0 = bass.AP(
                                tensor=ph1.tensor,
                                offset=ph1[:].offset,
                                ap=[[ph1[:].ap[0][0], 128], [0, 9], [1, NB]],
                            )
                            mm(out0, W1A[alpha], mv, start=False, stop=True,
                               skip_group_check=True)
                        else:
                            ai = {0.05: 0, 0.1: 1, DT / 6.0: 2}[alpha]
                            qt, qb, qr = qpair_prev[g]
                            if PUMP:
                                qv = qt[:].rearrange("p (k j) -> p k j", j=NB)
                                mm(rep_out(ph1, 2), W1A16[ai][0], qv,
                                   start=False, stop=False,
                                   skip_group_check=True)
                            else:
                                mm(ph1[:], W1A16[ai][0], qb, start=False,
                                   stop=False, skip_group_check=True)
                                mm(ph1[:], W1A16[ai][0], qr, start=False,
                                   stop=False, skip_group_check=True)
                            mm(ph1[:], W1A16[ai][1], qb, start=False,
                               stop=True, skip_group_check=True)

                    # ---- relu1 -> fp16 pair (one tile: value|residual)
                    h1t = hp.tile([128, 2 * NB], F16, tag=f"h1{g}")
                    hb1, hr1 = h1t[:, 0:NB], h1t[:, NB:2 * NB]
                    nc.vector.tensor_scalar_max(hb1, ph1[:], 0.0)
                    nc.vector.scalar_tensor_tensor(
                        out=hr1, in0=ph1[:], scalar=0.0, in1=hb1,
                        op0=OP.max, op1=OP.subtract)

                    # ---- L2 + b2 seed
                    ph2 = php.tile([128, NB], F32, tag=f"ph2g{g}")
                    mm(ph2[:], b2row2, onesr[:, 0:NB], start=True, stop=False,
                       skip_group_check=True)
                    if PUMP:
                        h1v = h1t[:].rearrange("p (k j) -> p k j", j=NB)
                        mm(rep_out(ph2, 2), w2b, h1v, start=False, stop=False,
                           skip_group_check=True)
                    else:
                        mm(ph2[:], w2b, hb1, start=False, stop=False,
                           skip_group_check=True)
                        mm(ph2[:], w2b, hr1, start=False, stop=False,
                           skip_group_check=True)
                    mm(ph2[:], w2r, hb1, start=False, stop=True,
                       skip_group_check=True)

                    # ---- relu2 -> fp16 pair (one tile)
                    h2t = hp.tile([128, 2 * NB], F16, tag=f"h2{g}")
                    hb2, hr2 = h2t[:, 0:NB], h2t[:, NB:2 * NB]
                    nc.vector.tensor_scalar_max(hb2, ph2[:], 0.0)
                    nc.vector.scalar_tensor_tensor(
                        out=hr2, in0=ph2[:], scalar=0.0, in1=hb2,
                        op0=OP.max, op1=OP.subtract)

                    # ---- L3: b3 seed + 8 chunks, stride-9 psum layout
                    gp = gp_full[:, g * W:(g + 1) * W]
                    gp3 = gp.rearrange("p (j c) -> p j c", c=9)
                    if STRIDE9_MM:
                        sel3 = sel16.rearrange("p (j c) -> p j c", c=9)[:, :, 0:8]
                        mm(gp3[:, :, 0:8], b3row16, sel3, start=True,
                           stop=False, skip_group_check=True)
                        h2v = h2t[:].rearrange("p (k j) -> p k j", j=NB)
                        for cc in range(8):
                            sl = gp3[:, :, cc]
                            last = cc == 7
                            wbc = w3b[:, cc * 128:(cc + 1) * 128]
                            wrc = w3r[:, cc * 128:(cc + 1) * 128]
                            if PUMP:
                                slr = bass.AP(
                                    tensor=gp_full.tensor,
                                    offset=sl.offset,
                                    ap=[[sl.ap[0][0], 128], [0, 2], [9, NB]],
                                )
                                mm(slr, wbc, h2v, start=False, stop=False,
                                   skip_group_check=True)
                            else:
                                mm(sl, wbc, hb2, start=False, stop=False,
                                   skip_group_check=True)
                                mm(sl, wbc, hr2, start=False, stop=False,
                                   skip_group_check=True)
                            mm(sl, wrc, hb2, start=False, stop=last,
                               skip_group_check=True)
                        nc.scalar.activation(
                            ge[:].rearrange("p (j c) -> p j c", c=9)[:, :, 0:8],
                            gp3[:, :, 0:8], AF.Tanh, bias=0.0)
                    else:
                        # chunk-contiguous psum; tanh output does the reorder
                        sel3 = sel16c
                        mm(gp[:, 0:64], b3row16, sel3, start=True,
                           stop=False, skip_group_check=True)
                        for cc in range(8):
                            sl = gp[:, cc * NB:(cc + 1) * NB]
                            last = cc == 7
                            mm(sl, w3b[:, cc * 128:(cc + 1) * 128], hb2,
                               start=False, stop=False, skip_group_check=True)
                            mm(sl, w3b[:, cc * 128:(cc + 1) * 128], hr2,
                               start=False, stop=False, skip_group_check=True)
                            mm(sl, w3r[:, cc * 128:(cc + 1) * 128], hb2,
                               start=False, stop=last, skip_group_check=True)
                        nc.scalar.activation(
                            ge[:].rearrange("p (j c) -> p c j", c=9)[:, 0:8, :],
                            gp[:, 0:64].rearrange("p (c j) -> p c j", j=NB),
                            AF.Tanh, bias=0.0)

                    # ---- einsum product tensor
                    mt = mp.tile([128, W], F32, tag=f"m{g}")
                    meng = nc.gpsimd if POOLMULT else nc.vector
                    meng.tensor_tensor(out=mt[:], in0=ge[:], in1=wfg,
                                       op=OP.mult)
                    m_prev[g] = mt

                    # ---- off-chain bookkeeping (gpsimd)
                    if DEBUG and t == 0 and g == 0:
                        if s == 0:
                            nc.sync.dma_start(out=dbg_h1[0], in_=hb1)
                            nc.sync.dma_start(out=dbg_h1[1], in_=hr1)
                            nc.sync.dma_start(out=dbg_h2[0], in_=hb2)
                            nc.sync.dma_start(out=dbg_h2[1], in_=hr2)
                            nc.sync.dma_start(out=dbg_ge, in_=ge[:])
                            nc.sync.dma_start(out=dbg_m, in_=mt[:])
                    q = qp.tile([128, NB], F32, tag=f"q{g}")
                    nc.vector.tensor_reduce(
                        out=q[:], in_=mt[:].rearrange("p (j c) -> p j c", c=9),
                        axis=mybir.AxisListType.X, op=OP.add)
                    qs[g][s] = q
                    q_prev[g] = q
                    if DEBUG and t == 0 and g == 0:
                        nc.sync.dma_start(out=dbg_q[s], in_=q[:])
                    if not DIRECT_MM:
                        qpair_prev[g] = pair_casts(g, q, "qp")

                    if s < 3:
                        # next stage's ge with svd col = z_{s+1}
                        ge_n = gep.tile([128, W], F32, tag=f"ge{g}")
                        ge_n3 = ge_n[:].rearrange("p (j c) -> p j c", c=9)
                        nc.vector.scalar_tensor_tensor(
                            out=ge_n3[:, :, 8], in0=q[:], scalar=ALPHAS[s],
                            in1=slot[g][:], op0=OP.mult, op1=OP.add)
                        ge_cur[g] = ge_n

                    if s == 1:
                        ka = qp.tile([128, NB], F32, tag=f"ka{g}")
                        nc.vector.scalar_tensor_tensor(
                            out=ka[:], in0=q[:], scalar=2.0, in1=qs[g][0][:],
                            op0=OP.mult, op1=OP.add)
                        kacc[g] = ka
                    elif s == 2:
                        ka2 = qp.tile([128, NB], F32, tag=f"ka{g}")
                        nc.vector.scalar_tensor_tensor(
                            out=ka2[:], in0=q[:], scalar=2.0, in1=kacc[g][:],
                            op0=OP.mult, op1=OP.add)
                        pf = sp.tile([128, NB], F32, tag=f"pfin{g}")
                        nc.vector.scalar_tensor_tensor(
                            out=pf[:], in0=ka2[:], scalar=DT / 6.0,
                            in1=slot[g][:], op0=OP.mult, op1=OP.add)
                        pfin[g] = pf
                        pft[g], pfb[g], pfr[g] = pair_casts(g, pf, "pf")
                    elif s == 3:
                        s_n = sp.tile([128, NB], F32, tag=f"slot{g}")
                        nc.vector.scalar_tensor_tensor(
                            out=s_n[:], in0=q[:], scalar=DT / 6.0,
                            in1=pfin[g][:], op0=OP.mult, op1=OP.add)
                        slot[g] = s_n
                        zft[g], zfb[g], zfr[g] = pair_casts(g, s_n, "zf")
                        ge_n = gep.tile([128, W], F32, tag=f"ge{g}")
                        ge_n3 = ge_n[:].rearrange("p (j c) -> p j c", c=9)
                        nc.vector.tensor_copy(out=ge_n3[:, :, 8], in_=s_n[:])
                        ge_cur[g] = ge_n

            pft_prev = list(pft)
            pfb_prev = list(pfb)
            pfr_prev = list(pfr)

    nc.compile()
    return nc


def _split16(w):
    wb = np.asarray(w, np.float32).astype(np.float16)
    wr = (np.asarray(w, np.float32) - wb.astype(np.float32)).astype(np.float16)
    return wb, wr


def _prep_inputs(t, x, dyn_w1, dyn_b1, dyn_w2, dyn_b2, dyn_w3, dyn_b3,
                 init_w1, init_b1, init_w2, init_b2, init_w3, init_b3,
                 l_steps=L):
    x = np.asarray(x, dtype=np.float32)
    x_aug = np.concatenate([x, x[:, -1:]], axis=1)
    v = (x_aug[:, 1:] - x_aug[:, :-1]) / DT  # [B, L, X]
    sv = v.sum(-1)  # [B, L]

    # W3 x-major permute
    w3x = np.empty((H, 1024), dtype=np.float32)
    for c in range(8):
        w3x[:, c * 128:(c + 1) * 128] = dyn_w3[:, ORIG_COL[c]]
    b3row = np.asarray(dyn_b3, np.float32)[ORIG_COL]  # [8, 128]

    w2bs, w2rs = _split16(dyn_w2)
    w3bs, w3rs = _split16(w3x)
    b3bs, b3rs = _split16(b3row)
    b1bs, b1rs = _split16(np.asarray(dyn_b1, np.float32).reshape(1, 128))
    b2bs, b2rs = _split16(np.asarray(dyn_b2, np.float32).reshape(1, 128))

    # w1dup fp16 pair [128, 256]
    w1dup = np.concatenate([dyn_w1, dyn_w1], axis=0).astype(np.float32)
    w1dbs, w1drs = _split16(w1dup)

    # w1dup*alpha fp32 [128, 384]
    w1da = np.concatenate([w1dup * 0.05, w1dup * 0.1, w1dup * (DT / 6.0)],
                          axis=1).astype(np.float32)
    # fp16 pairs of w1dup*alpha [128, 768]
    parts = []
    for a in (0.05, 0.1, DT / 6.0):
        ab, ar = _split16(w1dup * a)
        parts += [ab, ar]
    w1da16 = np.concatenate(parts, axis=1)

    # b12 [2, 256]: rows (value, residual) x cols (b1 | b2)
    b12 = np.zeros((2, 256), dtype=np.float16)
    b12[0, 0:128] = b1bs[0]
    b12[1, 0:128] = b1rs[0]
    b12[0, 128:256] = b2bs[0]
    b12[1, 128:256] = b2rs[0]

    # sel16 [16, 72]: sel16[k, j*9+c] = 1 if c == k%8 (c<8)
    sel16 = np.zeros((16, 72), dtype=np.float16)
    for k in range(16):
        for j in range(NB):
            sel16[k, j * 9 + (k % 8)] = 1.0
    b3row16 = np.concatenate([b3bs, b3rs], axis=0)  # [16, 128]
    sel16c = np.zeros((16, 64), dtype=np.float16)
    for k in range(16):
        for j in range(NB):
            sel16c[k, (k % 8) * NB + j] = 1.0
    b3s = np.concatenate([b3row16, sel16, sel16c], axis=1)  # [16, 264]

    wmm = np.concatenate([w1dbs, w1drs, w2bs, w2rs, w3bs, w3rs], axis=1)
    ones16 = np.ones((2, 16), dtype=np.float16)

    shared = dict(
        wmm=np.ascontiguousarray(wmm),
        w1da=np.ascontiguousarray(w1da),
        w1da16=np.ascontiguousarray(w1da16),
        b12=np.ascontiguousarray(b12),
        b3s=np.ascontiguousarray(b3s),
        onesr=ones16,
        wi2=np.asarray(init_w2, np.float32),
        wi3=np.asarray(init_w3, np.float32),
        bi1=np.asarray(init_b1, np.float32).reshape(128, 1),
        bi2=np.asarray(init_b2, np.float32).reshape(128, 1),
        bi3=np.asarray(init_b3, np.float32).reshape(64, 1),
    )
    wi1 = np.asarray(init_w1, np.float32)

    # wf [l, 128, 144]: wf[t, p, (g, j, c)] with 16 j-cols (j,c)-major,
    # groups at col offsets g*72.  c<8: dt*v[b, t, x(c,p)] ; c=8: svd scalar
    xs = 2 * _c[:, None] + (_p[None, :] // 64)  # [8, 128] x index per (c, p)
    in_maps = []
    for core in range(NCORES):
        bsl = slice(core * BPC, (core + 1) * BPC)
        vb = v[bsl, :l_steps]            # [16, l, X]
        svb = sv[bsl, :l_steps]          # [16, l]
        wf = np.empty((l_steps, 128, 144), dtype=np.float32)
        for j16 in range(16):
            gg, jj = j16 // NB, j16 % NB
            col0 = gg * W + jj * 9
            # [l, 8, 128] = dt * v[b, t, xs[c,p]]
            wf[:, :, col0:col0 + 8] = vb[j16][:, xs].transpose(0, 2, 1)
            wf[:, :, col0 + 8] = (-0.001 * svb[j16])[:, None]
        x0tc = x[bsl, 0, :].T.astype(np.float32)          # [X, 16]
        wi1x = np.concatenate([wi1, x0tc], axis=1)        # [16, 144]
        mdl = dict(shared)
        mdl.update(wf=np.ascontiguousarray(wf),
                   wi1x=np.ascontiguousarray(wi1x))
        in_maps.append(mdl)
    return in_maps


_NC_CACHE = {}


def kernel_traced(trace=False, **inputs):
    key = L
    if key not in _NC_CACHE:
        _NC_CACHE[key] = build_nc(L)
    nc = _NC_CACHE[key]
    in_maps = _prep_inputs(**inputs, l_steps=L)
    res = run_bass_kernel_spmd(nc, in_maps, list(range(NCORES)), trace=trace)
    out = np.empty((B, L, Z), dtype=np.float32)
    for core in range(NCORES):
        for g in range(G):
            zall = res.results[core][f"zall{g}"]  # [L, 128, NB]
            zf = zall[:, :Z] + zall[:, Z:]
            out[core * BPC + g * NB:core * BPC + (g + 1) * NB] = \
                zf.transpose(2, 0, 1)
    return out, res


def kernel(**inputs):
    return kernel_traced(trace=False, **inputs)[0]


# revision 5
# speedup vs baseline: 1.0767x; 1.0767x over previous
"""Neural CDE (RK4) Trainium2 kernel, v2.

Changes vs v1 baseline:
- double-fp16 (value+residual) weights/activations, 3-term products
  (~10x more accurate than double-bf16-4-term, fewer matmuls).
- G=2 interleaved batch-group pipelines per core (8+8 of 16 rows) so the
  two serial RK4 dependency chains fill each other's latency bubbles.
- "direct-MM": the x-contraction of the einsum is folded into L1's
  matmul via a stride-0 PSUM output AP (fp32 exact), removing the
  reduce + state-cast ops from the critical chain.
- L1 bias folded into an augmented [w1;b1] stationary (K=65) acting on
  folded state; L2/L3 biases via cheap fp16 row-seed matmuls.
- relu outputs fp16 directly on Act; fp16 residuals via one DVE stt from
  PSUM; bookkeeping (reduce, state updates, kaccs) on GpSimd off-chain.
- single per-step wf DMA (host-precomputed [128,144] (j,c)-major tile),
  zall output DMAs issued from the Pool queue.
"""

import os
import sys
from contextlib import ExitStack

import numpy as np
import ml_dtypes

sys.path.insert(0, "/opt/trn_rl_repo")

import concourse.bass as bass
import concourse.tile as tile
from concourse import bacc
from concourse import mybir
from concourse.bass_utils import run_bass_kernel_spmd

B, L, X, Z, H = 128, 512, 16, 64, 128
NCORES = 8
BPC = B // NCORES  # 16
G = 2              # batch groups per core
NB = BPC // G      # 8
W = 9 * NB         # 72 wf/ge/m columns per group
DT = 0.1
F32 = mybir.dt.float32
F16 = mybir.dt.float16
AF = mybir.ActivationFunctionType
OP = mybir.AluOpType

DIRECT_MM = True
STRIDE9_MM = True
GPDMA = True
POOLMULT = False
DEBUG = False
F32R = False
PUMP = True

# x-major permutation: psum/ge position (p, c) holds original W3 column
# z*16+x with x = 2c + (p>=64), z = p%64  (same as v1)
_p = np.arange(128)
_c = np.arange(8)
ORIG_COL = (_p[None, :] % 64) * 16 + 2 * _c[:, None] + (_p[None, :] // 64)  # [8,128]

ALPHAS = [0.5 * DT, 0.5 * DT, DT, DT / 6.0]  # stage input scales + final


def build_nc(l_steps=L):
    nc = bacc.Bacc("TRN2")
    dp = nc.declare_dram_parameter

    # ---- DRAM parameters (per core) ----
    wf_d = dp("wf", [l_steps, 128, 144], F32, isOutput=False).ap()
    # fp16 packed weights: [w1dupb|w1dupr|w2b|w2r|w3b(1024)|w3r(1024)]
    wmm_d = dp("wmm", [128, 2560], F16, isOutput=False).ap()
    # w1dup * alpha fp32(/fp32r), 3 variants packed [128, 384]
    w1da_d = dp("w1da", [128, 384],
                mybir.dt.float32r if F32R else F32, isOutput=False).ap()
    # fp16 pairs of w1dup*alpha: 3 variants x (b|r) packed [128, 768]
    w1da16_d = dp("w1da16", [128, 768], F16, isOutput=False).ap()
    # stacked bias rows fp16: [2, b1pair(128) | b2pair(128)]
    b12_d = dp("b12", [2, 256], F16, isOutput=False).ap()
    # stacked b3 rows + sel16(stride9) + sel16(contig): [16, 128+72+64]
    b3s_d = dp("b3s", [16, 264], F16, isOutput=False).ap()
    ones_d = dp("onesr", [2, 16], F16, isOutput=False).ap()
    # init mlp (fp32)
    wi1x_d = dp("wi1x", [16, 144], F32, isOutput=False).ap()  # [wi1 | x0t]
    wi2_d = dp("wi2", [128, 128], F32, isOutput=False).ap()
    wi3_d = dp("wi3", [128, 64], F32, isOutput=False).ap()
    bi1_d = dp("bi1", [128, 1], F32, isOutput=False).ap()
    bi2_d = dp("bi2", [128, 1], F32, isOutput=False).ap()
    bi3_d = dp("bi3", [64, 1], F32, isOutput=False).ap()
    # split-form state per step per group; host folds halves
    zout = [dp(f"zall{g}", [l_steps, 128, NB], F32, isOutput=True).ap()
            for g in range(G)]
    if DEBUG:
        dbg_ph1 = dp("dbg_ph1", [4, 128, NB], F32, isOutput=True).ap()
        dbg_h1 = dp("dbg_h1", [2, 128, NB], F16, isOutput=True).ap()
        dbg_h2 = dp("dbg_h2", [2, 128, NB], F16, isOutput=True).ap()
        dbg_ge = dp("dbg_ge", [128, 9 * NB], F32, isOutput=True).ap()
        dbg_m = dp("dbg_m", [128, 9 * NB], F32, isOutput=True).ap()
        dbg_q = dp("dbg_q", [4, 128, NB], F32, isOutput=True).ap()

    with tile.TileContext(nc) as tc, ExitStack() as ctx:
        singles = ctx.enter_context(tc.tile_pool(name="singles", bufs=1))
        wfp = ctx.enter_context(tc.tile_pool(name="wfp", bufs=4))
        gep = ctx.enter_context(tc.tile_pool(name="gep", bufs=12))
        mp = ctx.enter_context(tc.tile_pool(name="mp", bufs=12))
        hp = ctx.enter_context(tc.tile_pool(name="hp", bufs=12))
        qp = ctx.enter_context(tc.tile_pool(name="qp", bufs=12))
        sp = ctx.enter_context(tc.tile_pool(name="sp", bufs=8))   # slot/pfin
        zfp = ctx.enter_context(tc.tile_pool(name="zfp", bufs=12))  # folded state
        ph1pool = ctx.enter_context(tc.tile_pool(name="ph1p", bufs=2,
                                                  space="PSUM"))
        ph2pool = ctx.enter_context(tc.tile_pool(name="ph2p", bufs=1,
                                                  space="PSUM"))
        gpp = ctx.enter_context(tc.tile_pool(name="gpp", bufs=2, space="PSUM"))

        def load(pool, ap):
            t = pool.tile(list(ap.shape), ap.dtype, tag=ap.tensor.name)
            nc.sync.dma_start(out=t[:], in_=ap)
            return t

        wmm = load(singles, wmm_d)
        w1db, w1dr = wmm[:, 0:128], wmm[:, 128:256]
        w2b, w2r = wmm[:, 256:384], wmm[:, 384:512]
        w3b, w3r = wmm[:, 512:1536], wmm[:, 1536:2560]
        w1da = load(singles, w1da_d)
        W1A = {0.05: w1da[:, 0:128], 0.1: w1da[:, 128:256],
               DT / 6.0: w1da[:, 256:384]}
        w1da16 = load(singles, w1da16_d)
        W1A16 = [(w1da16[:, i * 256:i * 256 + 128],
                  w1da16[:, i * 256 + 128:(i + 1) * 256]) for i in range(3)]
        b12 = load(singles, b12_d)
        b1row2, b2row2 = b12[:, 0:128], b12[:, 128:256]
        b3s = load(singles, b3s_d)
        b3row16, sel16 = b3s[:, 0:128], b3s[:, 128:200]
        sel16c = b3s[:, 200:264]
        onesr = load(singles, ones_d)
        wi1x = load(singles, wi1x_d)
        wi1, x0t = wi1x[:, 0:128], wi1x[:, 128:144]
        wi2 = load(singles, wi2_d)
        wi3 = load(singles, wi3_d)
        bi1 = load(singles, bi1_d)
        bi2 = load(singles, bi2_d)
        bi3 = load(singles, bi3_d)

        mm = nc.tensor.matmul

        def rep_out(ph, n_rep):
            """psum out AP writing the same NB cols n_rep times (accumulate)."""
            base = ph[:]
            return bass.AP(tensor=ph.tensor, offset=base.offset,
                           ap=[[base.ap[0][0], 128], [0, n_rep], [1, NB]])

        # ---- init MLP (fp32): z0 = mlp(x(t0)) for all 16 batch cols ----
        gi = gpp.tile([128, 144], F32, tag="gp")
        ph_i1 = gi[:, 0:BPC]
        mm(ph_i1, wi1, x0t, start=True, stop=True)
        hi1 = singles.tile([128, BPC], F32, tag="hi1")
        nc.scalar.activation(hi1[:], ph_i1, AF.Relu, bias=bi1[:])
        gi2 = gpp.tile([128, 144], F32, tag="gp")
        ph_i2 = gi2[:, 0:BPC]
        mm(ph_i2, wi2[:], hi1[:], start=True, stop=True)
        hi2 = singles.tile([128, BPC], F32, tag="hi2")
        nc.scalar.activation(hi2[:], ph_i2, AF.Relu, bias=bi2[:])
        gi3 = gpp.tile([128, 144], F32, tag="gp")
        ph_i3 = gi3[0:64, 0:BPC]
        mm(ph_i3, wi3[:], hi2[:], start=True, stop=True)
        z0full = singles.tile([128, BPC], F32, tag="z0full")
        nc.vector.tensor_scalar_add(z0full[0:64, :], ph_i3, bi3[:])
        nc.vector.memset(z0full[64:128, :], 0.0)

        # per-group recurrent state
        slot = [None] * G     # [128, NB] f32 split-form z_n
        zft = [None] * G      # [128, 2NB] f16 state pair tile
        zfb = [None] * G
        zfr = [None] * G
        ge_cur = [None] * G   # [128, W] f32, svd col pre-written
        m_prev = [None] * G   # [128, W] f32 last stage's product tensor
        q_prev = [None] * G   # [128, NB] f32 last stage's reduced k
        qpair_prev = [None] * G

        def pair_casts(g, src_slot, tag):
            """src_slot [128,NB] f32 -> fp16 (value|residual) in one tile."""
            ft = zfp.tile([128, 2 * NB], F16, tag=f"{tag}{g}")
            fb, fr = ft[:, 0:NB], ft[:, NB:2 * NB]
            nc.scalar.copy(out=fb, in_=src_slot[:])
            nc.vector.scalar_tensor_tensor(out=fr, in0=fb, scalar=-1.0,
                                           in1=src_slot[:], op0=OP.mult,
                                           op1=OP.add)
            return ft, fb, fr

        for g in range(G):
            s0 = sp.tile([128, NB], F32, tag=f"slot{g}")
            nc.vector.tensor_copy(out=s0[:], in_=z0full[:, g * NB:(g + 1) * NB])
            slot[g] = s0
            zft[g], zfb[g], zfr[g] = pair_casts(g, s0, "zf")
            geg = gep.tile([128, W], F32, tag=f"ge{g}")
            ge3 = geg[:].rearrange("p (j c) -> p j c", c=9)
            nc.vector.tensor_copy(out=ge3[:, :, 8], in_=s0[:])
            ge_cur[g] = geg

        for t in range(l_steps):
            wf = wfp.tile([128, 144], F32, tag="wf")
            nc.sync.dma_start(out=wf[:], in_=wf_d[t])

            for g in range(G):
                eng = nc.gpsimd if GPDMA else nc.sync
                eng.dma_start(out=zout[g][t], in_=slot[g][:])

            pfin = [None] * G
            pft = [None] * G
            pfb = [None] * G
            pfr = [None] * G
            kacc = [None] * G
            qs = [[None] * 4 for _ in range(G)]

            for s in range(4):
                gp_full = gpp.tile([128, 144], F32, tag="gp")
                for g in range(G):
                    wfg = wf[:, g * W:(g + 1) * W]
                    ge = ge_cur[g]

                    # ---- L1: base (split-form state) + b1 seed + direct MM
                    ph1 = ph1pool.tile([128, NB], F32, tag=f"ph1g{g}")
                    if s == 0 and t > 0:
                        bt, bb, br = pft_prev[g], pfb_prev[g], pfr_prev[g]
                    else:
                        bt, bb, br = zft[g], zfb[g], zfr[g]
                    mm(ph1[:], b1row2, onesr[:, 0:NB], start=True, stop=False,
                       skip_group_check=True)
                    last_base = t == 0 and s == 0
                    if PUMP:
                        mv2 = bt[:].rearrange("p (k j) -> p k j", j=NB)
                        mm(rep_out(ph1, 2), w1db, mv2, start=False, stop=False,
                           skip_group_check=True)
                    else:
                        mm(ph1[:], w1db, bb, start=False, stop=False,
                           skip_group_check=True)
                        mm(ph1[:], w1db, br, start=False, stop=False,
                           skip_group_check=True)
                    mm(ph1[:], w1dr, bb, start=False, stop=last_base,
                       skip_group_check=True)
                    if not last_base:
                        alpha = ALPHAS[s - 1] if s > 0 else DT / 6.0
                        if DIRECT_MM:
                            # rhs = m_prev viewed (c outer, j inner); out
                            # stride-0 over c accumulates the x-reduce.
                            mv = m_prev[g][:].rearrange("p (j c) -> p c j",
                                                        c=9)
                            if F32R:
                                mv = mv.bitcast(mybir.dt.float32r)
                            out0 = bass.AP(
                                tensor=ph1.tensor,
                                offset=ph1[:].offset,
                                ap=[[ph1[:].ap[0][0], 128], [0, 9], [1, NB]],
                            )
                            mm(out0, W1A[alpha], mv, start=False, stop=True,
                               skip_group_check=True)
                        else:
                            ai = {0.05: 0, 0.1: 1, DT / 6.0: 2}[alpha]
                            qt, qb, qr = qpair_prev[g]
                            if PUMP:
                                qv = qt[:].rearrange("p (k j) -> p k j", j=NB)
                                mm(rep_out(ph1, 2), W1A16[ai][0], qv,
                                   start=False, stop=False,
                                   skip_group_check=True)
                            else:
                                mm(ph1[:], W1A16[ai][0], qb, start=False,
                                   stop=False, skip_group_check=True)
                                mm(ph1[:], W1A16[ai][0], qr, start=False,
                                   stop=False, skip_group_check=True)
                            mm(ph1[:], W1A16[ai][1], qb, start=False,
                               stop=True, skip_group_check=True)

                    # ---- relu1 -> fp16 pair (one tile: value|residual)
                    h1t = hp.tile([128, 2 * NB], F16, tag=f"h1{g}")
                    hb1, hr1 = h1t[:, 0:NB], h1t[:, NB:2 * NB]
                    nc.scalar.activation(hb1, ph1[:], AF.Relu, bias=0.0)
                    nc.vector.scalar_tensor_tensor(
                        out=hr1, in0=ph1[:], scalar=0.0, in1=hb1,
                        op0=OP.max, op1=OP.subtract)

                    # ---- L2 + b2 seed
                    ph2 = ph2pool.tile([128, NB], F32, tag=f"ph2g{g}")
                    mm(ph2[:], b2row2, onesr[:, 0:NB], start=True, stop=False,
                       skip_group_check=True)
                    if PUMP:
                        h1v = h1t[:].rearrange("p (k j) -> p k j", j=NB)
                        mm(rep_out(ph2, 2), w2b, h1v, start=False, stop=False,
                           skip_group_check=True)
                    else:
                        mm(ph2[:], w2b, hb1, start=False, stop=False,
                           skip_group_check=True)
                        mm(ph2[:], w2b, hr1, start=False, stop=False,
                           skip_group_check=True)
                    mm(ph2[:], w2r, hb1, start=False, stop=True,
                       skip_group_check=True)

                    # ---- relu2 -> fp16 pair (one tile)
                    h2t = hp.tile([128, 2 * NB], F16, tag=f"h2{g}")
                    hb2, hr2 = h2t[:, 0:NB], h2t[:, NB:2 * NB]
                    nc.scalar.activation(hb2, ph2[:], AF.Relu, bias=0.0)
                    nc.vector.scalar_tensor_tensor(
                        out=hr2, in0=ph2[:], scalar=0.0, in1=hb2,
                        op0=OP.max, op1=OP.subtract)

                    # ---- L3: b3 seed + 8 chunks, stride-9 psum layout
                    gp = gp_full[:, g * W:(g + 1) * W]
                    gp3 = gp.rearrange("p (j c) -> p j c", c=9)
                    if STRIDE9_MM:
                        sel3 = sel16.rearrange("p (j c) -> p j c", c=9)[:, :, 0:8]
                        mm(gp3[:, :, 0:8], b3row16, sel3, start=True,
                           stop=False, skip_group_check=True)
                        h2v = h2t[:].rearrange("p (k j) -> p k j", j=NB)
                        for cc in range(8):
                            sl = gp3[:, :, cc]
                            last = cc == 7
                            wbc = w3b[:, cc * 128:(cc + 1) * 128]
                            wrc = w3r[:, cc * 128:(cc + 1) * 128]
                            if PUMP:
                                slr = bass.AP(
                                    tensor=gp_full.tensor,
                                    offset=sl.offset,
                                    ap=[[sl.ap[0][0], 128], [0, 2], [9, NB]],
                                )
                                mm(slr, wbc, h2v, start=False, stop=False,
                                   skip_group_check=True)
                            else:
                                mm(sl, wbc, hb2, start=False, stop=False,
                                   skip_group_check=True)
                                mm(sl, wbc, hr2, start=False, stop=False,
                                   skip_group_check=True)
                            mm(sl, wrc, hb2, start=False, stop=last,
                               skip_group_check=True)
                        nc.scalar.activation(
                            ge[:].rearrange("p (j c) -> p j c", c=9)[:, :, 0:8],
                            gp3[:, :, 0:8], AF.Tanh, bias=0.0)
                    else:
                        # chunk-contiguous psum; tanh output does the reorder
                        sel3 = sel16c
                        mm(gp[:, 0:64], b3row16, sel3, start=True,
                           stop=False, skip_group_check=True)
                        for cc in range(8):
                            sl = gp[:, cc * NB:(cc + 1) * NB]
                            last = cc == 7
                            mm(sl, w3b[:, cc * 128:(cc + 1) * 128], hb2,
                               start=False, stop=False, skip_group_check=True)
                            mm(sl, w3b[:, cc * 128:(cc + 1) * 128], hr2,
                               start=False, stop=False, skip_group_check=True)
                            mm(sl, w3r[:, cc * 128:(cc + 1) * 128], hb2,
                               start=False, stop=last, skip_group_check=True)
                        nc.scalar.activation(
                            ge[:].rearrange("p (j c) -> p c j", c=9)[:, 0:8, :],
                            gp[:, 0:64].rearrange("p (c j) -> p c j", j=NB),
                            AF.Tanh, bias=0.0)

                    # ---- einsum product tensor
                    mt = mp.tile([128, W], F32, tag=f"m{g}")
                    meng = nc.gpsimd if POOLMULT else nc.vector
                    meng.tensor_tensor(out=mt[:], in0=ge[:], in1=wfg,
                                       op=OP.mult)
                    m_prev[g] = mt

                    # ---- off-chain bookkeeping (gpsimd)
                    if DEBUG and t == 0 and g == 0:
                        if s == 0:
                            nc.sync.dma_start(out=dbg_h1[0], in_=hb1)
                            nc.sync.dma_start(out=dbg_h1[1], in_=hr1)
                            nc.sync.dma_start(out=dbg_h2[0], in_=hb2)
                            nc.sync.dma_start(out=dbg_h2[1], in_=hr2)
                            nc.sync.dma_start(out=dbg_ge, in_=ge[:])
                            nc.sync.dma_start(out=dbg_m, in_=mt[:])
                    q = qp.tile([128, NB], F32, tag=f"q{g}")
                    nc.vector.tensor_reduce(
                        out=q[:], in_=mt[:].rearrange("p (j c) -> p j c", c=9),
                        axis=mybir.AxisListType.X, op=OP.add)
                    qs[g][s] = q
                    q_prev[g] = q
                    if DEBUG and t == 0 and g == 0:
                        nc.sync.dma_start(out=dbg_q[s], in_=q[:])
                    if not DIRECT_MM:
                        qpair_prev[g] = pair_casts(g, q, "qp")

                    if s < 3:
                        # next stage's ge with svd col = z_{s+1}
                        ge_n = gep.tile([128, W], F32, tag=f"ge{g}")
                        ge_n3 = ge_n[:].rearrange("p (j c) -> p j c", c=9)
                        nc.vector.scalar_tensor_tensor(
                            out=ge_n3[:, :, 8], in0=q[:], scalar=ALPHAS[s],
                            in1=slot[g][:], op0=OP.mult, op1=OP.add)
                        ge_cur[g] = ge_n

                    if s == 1:
                        ka = qp.tile([128, NB], F32, tag=f"ka{g}")
                        nc.vector.scalar_tensor_tensor(
                            out=ka[:], in0=q[:], scalar=2.0, in1=qs[g][0][:],
                            op0=OP.mult, op1=OP.add)
                        kacc[g] = ka
                    elif s == 2:
                        ka2 = qp.tile([128, NB], F32, tag=f"ka{g}")
                        nc.vector.scalar_tensor_tensor(
                            out=ka2[:], in0=q[:], scalar=2.0, in1=kacc[g][:],
                            op0=OP.mult, op1=OP.add)
                        pf = sp.tile([128, NB], F32, tag=f"pfin{g}")
                        nc.vector.scalar_tensor_tensor(
                            out=pf[:], in0=ka2[:], scalar=DT / 6.0,
                            in1=slot[g][:], op0=OP.mult, op1=OP.add)
                        pfin[g] = pf
                        pft[g], pfb[g], pfr[g] = pair_casts(g, pf, "pf")
                    elif s == 3:
                        s_n = sp.tile([128, NB], F32, tag=f"slot{g}")
                        nc.vector.scalar_tensor_tensor(
                            out=s_n[:], in0=q[:], scalar=DT / 6.0,
                            in1=pfin[g][:], op0=OP.mult, op1=OP.add)
                        slot[g] = s_n
                        zft[g], zfb[g], zfr[g] = pair_casts(g, s_n, "zf")
                        ge_n = gep.tile([128, W], F32, tag=f"ge{g}")
                        ge_n3 = ge_n[:].rearrange("p (j c) -> p j c", c=9)
                        nc.vector.tensor_copy(out=ge_n3[:, :, 8], in_=s_n[:])
                        ge_cur[g] = ge_n

            pft_prev = list(pft)
            pfb_prev = list(pfb)
            pfr_prev = list(pfr)

    nc.compile()
    return nc


def _split16(w):
    wb = np.asarray(w, np.float32).astype(np.float16)
    wr = (np.asarray(w, np.float32) - wb.astype(np.float32)).astype(np.float16)
    return wb, wr


def _prep_inputs(t, x, dyn_w1, dyn_b1, dyn_w2, dyn_b2, dyn_w3, dyn_b3,
                 init_w1, init_b1, init_w2, init_b2, init_w3, init_b3,
                 l_steps=L):
    x = np.asarray(x, dtype=np.float32)
    x_aug = np.concatenate([x, x[:, -1:]], axis=1)
    v = (x_aug[:, 1:] - x_aug[:, :-1]) / DT  # [B, L, X]
    sv = v.sum(-1)  # [B, L]

    # W3 x-major permute
    w3x = np.empty((H, 1024), dtype=np.float32)
    for c in range(8):
        w3x[:, c * 128:(c + 1) * 128] = dyn_w3[:, ORIG_COL[c]]
    b3row = np.asarray(dyn_b3, np.float32)[ORIG_COL]  # [8, 128]

    w2bs, w2rs = _split16(dyn_w2)
    w3bs, w3rs = _split16(w3x)
    b3bs, b3rs = _split16(b3row)
    b1bs, b1rs = _split16(np.asarray(dyn_b1, np.float32).reshape(1, 128))
    b2bs, b2rs = _split16(np.asarray(dyn_b2, np.float32).reshape(1, 128))

    # w1dup fp16 pair [128, 256]
    w1dup = np.concatenate([dyn_w1, dyn_w1], axis=0).astype(np.float32)
    w1dbs, w1drs = _split16(w1dup)

    # w1dup*alpha fp32 [128, 384]
    w1da = np.concatenate([w1dup * 0.05, w1dup * 0.1, w1dup * (DT / 6.0)],
                          axis=1).astype(np.float32)
    # fp16 pairs of w1dup*alpha [128, 768]
    parts = []
    for a in (0.05, 0.1, DT / 6.0):
        ab, ar = _split16(w1dup * a)
        parts += [ab, ar]
    w1da16 = np.concatenate(parts, axis=1)

    # b12 [2, 256]: rows (value, residual) x cols (b1 | b2)
    b12 = np.zeros((2, 256), dtype=np.float16)
    b12[0, 0:128] = b1bs[0]
    b12[1, 0:128] = b1rs[0]
    b12[0, 128:256] = b2bs[0]
    b12[1, 128:256] = b2rs[0]

    # sel16 [16, 72]: sel16[k, j*9+c] = 1 if c == k%8 (c<8)
    sel16 = np.zeros((16, 72), dtype=np.float16)
    for k in range(16):
        for j in range(NB):
            sel16[k, j * 9 + (k % 8)] = 1.0
    b3row16 = np.concatenate([b3bs, b3rs], axis=0)  # [16, 128]
    sel16c = np.zeros((16, 64), dtype=np.float16)
    for k in range(16):
        for j in range(NB):
            sel16c[k, (k % 8) * NB + j] = 1.0
    b3s = np.concatenate([b3row16, sel16, sel16c], axis=1)  # [16, 264]

    wmm = np.concatenate([w1dbs, w1drs, w2bs, w2rs, w3bs, w3rs], axis=1)
    ones16 = np.ones((2, 16), dtype=np.float16)

    shared = dict(
        wmm=np.ascontiguousarray(wmm),
        w1da=np.ascontiguousarray(w1da),
        w1da16=np.ascontiguousarray(w1da16),
        b12=np.ascontiguousarray(b12),
        b3s=np.ascontiguousarray(b3s),
        onesr=ones16,
        wi2=np.asarray(init_w2, np.float32),
        wi3=np.asarray(init_w3, np.float32),
        bi1=np.asarray(init_b1, np.float32).reshape(128, 1),
        bi2=np.asarray(init_b2, np.float32).reshape(128, 1),
        bi3=np.asarray(init_b3, np.float32).reshape(64, 1),
    )
    wi1 = np.asarray(init_w1, np.float32)

    # wf [l, 128, 144]: wf[t, p, (g, j, c)] with 16 j-cols (j,c)-major,
    # groups at col offsets g*72.  c<8: dt*v[b, t, x(c,p)] ; c=8: svd scalar
    xs = 2 * _c[:, None] + (_p[None, :] // 64)  # [8, 128] x index per (c, p)
    in_maps = []
    for core in range(NCORES):
        bsl = slice(core * BPC, (core + 1) * BPC)
        vb = v[bsl, :l_steps]            # [16, l, X]
        svb = sv[bsl, :l_steps]          # [16, l]
        wf = np.empty((l_steps, 128, 144), dtype=np.float32)
        for j16 in range(16):
            gg, jj = j16 // NB, j16 % NB
            col0 = gg * W + jj * 9
            # [l, 8, 128] = dt * v[b, t, xs[c,p]]
            wf[:, :, col0:col0 + 8] = vb[j16][:, xs].transpose(0, 2, 1)
            wf[:, :, col0 + 8] = (-0.001 * svb[j16])[:, None]
        x0tc = x[bsl, 0, :].T.astype(np.float32)          # [X, 16]
        wi1x = np.concatenate([wi1, x0tc], axis=1)        # [16, 144]
        mdl = dict(shared)
        mdl.update(wf=np.ascontiguousarray(wf),
                   wi1x=np.ascontiguousarray(wi1x))
        in_maps.append(mdl)
    return in_maps


_NC_CACHE = {}


def kernel_traced(trace=False, **inputs):
    key = L
    if key not in _NC_CACHE:
        _NC_CACHE[key] = build_nc(L)
    nc = _NC_CACHE[key]
    in_maps = _prep_inputs(**inputs, l_steps=L)
    res = run_bass_kernel_spmd(nc, in_maps, list(range(NCORES)), trace=trace)
    out = np.empty((B, L, Z), dtype=np.float32)
    for core in range(NCORES):
        for g in range(G):
            zall = res.results[core][f"zall{g}"]  # [L, 128, NB]
            zf = zall[:, :Z] + zall[:, Z:]
            out[core * BPC + g * NB:core * BPC + (g + 1) * NB] = \
                zf.transpose(2, 0, 1)
    return out, res


def kernel(**inputs):
    return kernel_traced(trace=False, **inputs)[0]
